# revision 51
# baseline (speedup 1.0000x reference)
"""Trainium2 Bass kernel for nn_BoneRefusion (17-group BoneMLP over [B,T,16,3]).

Strategy (pure data parallel over batch, 8 cores):
  - Host pre-packs per-core inputs feature-major in a 2-set layout:
      x2 [98, S] bf16, S = tokens_per_core/2.
      Rows 0-47 = 48 features (16 bones x 3 coords) of token set A (first
      half), row 48 = ones (bakes b1 into layer 1), rows 49-96 = set B,
      row 97 = ones. Column j holds the token pair (A_j, B_j).
  - Groups 0-15 run on the device; group 16 (the ragged 17th, a 6->16->3
    MLP = ~6% of the FLOPs) is computed on the host in numpy, which makes
    the device schedule perfectly regular: every matmul is a 32-PE-column
    unit with identical tile config (128,32), 20 units per block = exactly
    5 rounds of N=512 across the four PE column groups.
  - Layer 1 (h = relu(x @ W1 + b1)): 16 column units (4 passes x 4).
  - Layer 2 (out = h @ W2 + b2): four 32-column streams, software-pipelined
    one block behind L1 so their semaphore waits are long satisfied.
  - PSUM: pass-0/1 banks are single but evacuated in short per-pass ops;
    the pass-2/3 pair and the L2 bank are double-buffered. This breaks
    every write-after-read cycle that previously paced the pipeline.
  - The out evacuation alternates ACT/DVE by block parity to balance the
    two PSUM-capable engines.
  - Output leaves the device feature-major in bf16 (tolerance is 2e-2;
    measured error ~2.9e-3); the host transposes/casts back to f32.
"""

import sys

import numpy as np
import ml_dtypes

sys.path.insert(0, "/opt/trn_rl_repo")

import concourse.bass as bass
import concourse.mybir as mybir
import concourse.tile as tile
from concourse import bacc
from concourse.bass_utils import run_bass_kernel_spmd

BF16 = mybir.dt.bfloat16
F32 = mybir.dt.float32
BF16_NP = ml_dtypes.bfloat16

LIMBS = [[0, 1, 2], [3, 4, 5], [6, 7], [8, 9], [10, 11, 12], [13, 14, 15],
         [6, 7, 1, 2], [6, 7, 4, 5], [6, 7, 11, 12], [6, 7, 14, 15], [6, 7, 9],
         [14, 15, 11, 12], [1, 2, 4, 5], [14, 15, 4, 5], [11, 12, 4, 5],
         [10, 0], [13, 3]]
NG = 17          # groups
HID = 16         # hidden per group
B, T, NJ, C = 2048, 243, 16, 3
NF = NJ * C      # 48 input features per token
NCORES = 8
BC = B // NCORES           # batches per core
TC = BC * T                # tokens per core
S = TC // 2                # token pairs per core (2-set packing)
KX = 2 * (NF + 1)          # 98: two sets of (48 features + ones row)
NBLK = 512                 # token-pairs per block (psum free dim)
NB = (S + NBLK - 1) // NBLK   # 61 blocks (60x512 + 1x384)

# L2 stream order across PSUM quarters of the `op` bank: stream q covers
# GROUPS_L2[q], reading h of L1 pass PASS_OF_STREAM[q] from the prev block.
GROUPS_L2 = [(12, 4), (0, 4), (4, 4), (8, 4)]
PASS_OF_STREAM = [3, 0, 1, 2]


def _host_weights(W1, b1, W2, b2, idx):
    """Build stationary operands + evac bias vectors on the host.

    Returns (wsb [128, 640] bf16, bsb [128, 1] f32).
      wsb cols 0-511: L1 passes 0-3 ([98,128] each: rows 0-47 set A block,
        row 48 = set A b1, rows 49-96 set B block, row 97 = set B b1).
      wsb cols 512-639: L2 streams q=0..3 ([128,32] each).
      bsb col 0: b2 for the L2 psum bank (per-partition).
    """
    W1 = np.asarray(W1, np.float32)
    b1 = np.asarray(b1, np.float32)
    W2 = np.asarray(W2, np.float32)
    b2 = np.asarray(b2, np.float32)
    idx = np.asarray(idx)

    # Scatter per-group [12, 16] W1 blocks into the 48-feature space.
    # Padded limb rows of W1 are already zero, so += handles duplicates.
    w1full = np.zeros((NF, NG * HID), np.float32)
    for g in range(NG):
        for j in range(4):
            r = int(idx[g, j]) * C
            w1full[r:r + C, g * HID:(g + 1) * HID] += W1[g, j * C:(j + 1) * C, :]
    b1flat = b1.reshape(NG * HID)

    wsb = np.zeros((128, 640), np.float32)
    for w in range(4):
        blk = w1full[:, 64 * w:64 * w + 64]            # [48, 64]
        bias = b1flat[64 * w:64 * w + 64]
        wsb[0:NF, 128 * w:128 * w + 64] = blk          # set A
        wsb[NF, 128 * w:128 * w + 64] = bias
        wsb[NF + 1:2 * NF + 1, 128 * w + 64:128 * w + 128] = blk   # set B
        wsb[2 * NF + 1, 128 * w + 64:128 * w + 128] = bias
    for q, (g0, ng) in enumerate(GROUPS_L2):
        col = 512 + 32 * q
        for j in range(ng):
            g = g0 + j
            wsb[16 * j:16 * j + 16, col + 3 * j:col + 3 * j + 3] = W2[g]
            wsb[64 + 16 * j:64 + 16 * j + 16,
                col + 12 + 3 * j:col + 12 + 3 * j + 3] = W2[g]

    bsb = np.zeros((128, 1), np.float32)
    for q, (g0, ng) in enumerate(GROUPS_L2):
        v = b2[g0:g0 + ng].reshape(-1)                 # 12 values
        bsb[32 * q:32 * q + 12, 0] = v
        bsb[32 * q + 12:32 * q + 24, 0] = v

    return wsb.astype(BF16_NP), bsb


def _build_nc():
    nc = bacc.Bacc(
        "TRN2", target_bir_lowering=False, debug=False, num_devices=NCORES,
    )
    x2 = nc.dram_tensor("x2", [KX, S], BF16, kind="ExternalInput").ap()
    wsd = nc.dram_tensor("wsd", [128, 640], BF16, kind="ExternalInput").ap()
    bsd = nc.dram_tensor("bsd", [128, 1], F32, kind="ExternalInput").ap()
    # Device output, feature-major bf16 = the L2 psum bank layout
    # (quarter q rows 32q..32q+24 real, 12 set-A cols then 12 set-B).
    outd = nc.dram_tensor("outd", [128, S], BF16, kind="ExternalOutput").ap()

    with tile.TileContext(nc) as tc:
        with (
            tc.tile_pool(name="singles", bufs=1) as singles,
            tc.tile_pool(name="xin", bufs=4) as xin,
            tc.tile_pool(name="hsb", bufs=2) as hsb,
            tc.tile_pool(name="osb", bufs=3) as osb,
            tc.tile_pool(name="hps", bufs=1, space="PSUM") as hps,
            tc.tile_pool(name="ops", bufs=1, space="PSUM") as opsp,
        ):
            ws = singles.tile([128, 640], BF16)
            nc.sync.dma_start(ws, wsd)
            bs = singles.tile([128, 1], F32)
            nc.sync.dma_start(bs, bsd)

            h_prev = None       # (h0s, h1s, h23s) sbuf tiles of prev block
            nb_prev = 0
            xtd = None

            for s in range(NB + 1):
                cur = s if s < NB else None
                prev = s - 1 if s >= 1 else None
                if cur is not None:
                    off = cur * NBLK
                    nb = min(NBLK, S - off)
                    if s % 2 == 0:
                        # one DMA covers this block and the next
                        ld = min(2 * NBLK, S - off)
                        xtd = xin.tile([KX, 2 * NBLK], BF16, tag="xt")
                        nc.sync.dma_start(xtd[:, :ld], x2[:, off:off + ld])
                        xt = xtd[:, 0:NBLK]
                    else:
                        xt = xtd[:, NBLK:2 * NBLK]

                # ---- PE: 32-col units, round-robin over column groups ----
                if cur is not None:
                    # DVE evacuates passes 1,2; ACT passes 0,3. Emitting
                    # passes in [1,2,0,3] staggers the evac ready-times
                    # across both engines; late-evacuated banks (hp2, hp3)
                    # are double-buffered.
                    hp = [
                        hps.tile([128, NBLK], F32, tag="hp0", name="hp0"),
                        hps.tile([128, NBLK], F32, tag="hp1", name="hp1"),
                        hps.tile([128, NBLK], F32, tag="hp2", name="hp2",
                                 bufs=2),
                        hps.tile([128, NBLK], F32, tag="hp3", name="hp3",
                                 bufs=2),
                    ]
                    for w in [1, 2, 0, 3]:
                        for j in range(4):
                            nc.tensor.matmul(
                                hp[w][32 * j:32 * j + 32, :nb],
                                lhsT=ws[0:KX,
                                        128 * w + 32 * j:128 * w + 32 * j + 32],
                                rhs=xt[:, :nb],
                                start=True, stop=True,
                                tile_position=(0, 32 * j),
                            )
                if prev is not None:
                    rhs_of = [h_prev[3], h_prev[0], h_prev[1], h_prev[2]]
                    op = opsp.tile([128, NBLK], F32, tag="op", bufs=2)
                    # the stream reading the latest-evacuated pass goes last
                    for q in [1, 2, 3, 0]:
                        nc.tensor.matmul(
                            op[32 * q:32 * q + 32, :nb_prev],
                            lhsT=ws[0:128, 512 + 32 * q:512 + 32 * q + 32],
                            rhs=rhs_of[q][:, :nb_prev],
                            start=True, stop=True,
                            tile_position=(0, 32 * q),
                        )

                # ---- evacuations ----
                if cur is not None:
                    hts = [hsb.tile([128, NBLK], BF16, tag=f"h{w}",
                                    name=f"h{w}") for w in range(4)]
                    nc.vector.tensor_scalar(
                        hts[1][:, :nb], hp[1][:, :nb], 0.0, None,
                        mybir.AluOpType.max,
                    )
                    nc.scalar.activation(
                        out=hts[0][:, :nb], in_=hp[0][:, :nb],
                        func=mybir.ActivationFunctionType.Relu,
                    )
                    nc.vector.tensor_scalar(
                        hts[2][:, :nb], hp[2][:, :nb], 0.0, None,
                        mybir.AluOpType.max,
                    )
                    nc.scalar.activation(
                        out=hts[3][:, :nb], in_=hp[3][:, :nb],
                        func=mybir.ActivationFunctionType.Relu,
                    )
                else:
                    hts = None
                if prev is not None:
                    ost = osb.tile([128, NBLK], BF16, tag="os")
                    if s % 2 == 0:
                        nc.vector.tensor_scalar(
                            ost[:, :nb_prev], op[:, :nb_prev],
                            bs[:, 0:1], None, mybir.AluOpType.add,
                        )
                    else:
                        nc.scalar.activation(
                            out=ost[:, :nb_prev], in_=op[:, :nb_prev],
                            func=mybir.ActivationFunctionType.Identity,
                            bias=bs[:, 0:1], scale=1.0,
                        )
                    offp = prev * NBLK
                    nc.scalar.dma_start(
                        outd[0:128, offp:offp + nb_prev], ost[:, :nb_prev])

                h_prev = hts
                if cur is not None:
                    nb_prev = nb
    nc.finalize()
    return nc


_NC_CACHE = None


def _get_nc():
    global _NC_CACHE
    if _NC_CACHE is None:
        _NC_CACHE = _build_nc()
    return _NC_CACHE


# outd row map (see _build_nc): stream q of GROUPS_L2 at rows 32q..32q+24
# (12 set-A cols then 12 set-B).  In group order 0..15:
_ROWS_A = np.r_[32:44, 64:76, 96:108, 0:12]
_ROWS_B = np.r_[44:56, 76:88, 108:120, 12:24]


def _g16_host(x, W1, b1, W2, b2, idx):
    """Group 16 (a 6->16->3 MLP, ~6% of the FLOPs) in numpy fp32."""
    g = x[:, :, idx[16], :].reshape(B, T, 4 * C)[:, :, :2 * C]
    h = np.maximum(g @ W1[16, :2 * C].astype(np.float32)
                   + b1[16].astype(np.float32), 0.0)
    return h @ W2[16].astype(np.float32) + b2[16].astype(np.float32)


def _kernel_impl(x, W1, b1, W2, b2, idx, _want_trace=False):
    x = np.asarray(x, np.float32)
    idx = np.asarray(idx)
    wsb, bsb = _host_weights(W1, b1, W2, b2, idx)

    in_maps = []
    for c in range(NCORES):
        xc = x[c * BC:(c + 1) * BC].reshape(TC, NF)
        xt2 = np.empty((KX, S), BF16_NP)
        xt2[0:NF] = np.ascontiguousarray(xc[:S].T)
        xt2[NF] = np.float32(1.0)
        xt2[NF + 1:2 * NF + 1] = np.ascontiguousarray(xc[S:].T)
        xt2[2 * NF + 1] = np.float32(1.0)
        in_maps.append({"x2": xt2, "wsd": wsb, "bsd": bsb})

    nc = _get_nc()
    res = run_bass_kernel_spmd(
        nc, in_maps, core_ids=list(range(NCORES)), trace=_want_trace,
    )

    out = np.empty((B, T, NG, C), np.float32)
    out[:, :, 16, :] = _g16_host(x, W1, b1, W2, b2, idx)
    for c in range(NCORES):
        od = np.asarray(res.results[c]["outd"], dtype=np.float32)  # [128, S]
        oc = np.empty((TC, 16 * C), np.float32)
        oc[:S] = od[_ROWS_A].T
        oc[S:] = od[_ROWS_B].T
        out[c * BC:(c + 1) * BC, :, 0:16, :] = oc.reshape(BC, T, 16, C)
    return out, res


def kernel(**inputs):
    out, _ = _kernel_impl(**inputs)
    return out


# revision 54
# speedup vs baseline: 1.2687x; 1.2687x over previous
"""Trainium2 Bass kernel for nn_BoneRefusion (17-group BoneMLP over [B,T,16,3]).

Strategy (pure data parallel over batch, 8 cores):
  - Host pre-packs per-core inputs feature-major in a 2-set layout:
      x2 [98, S] bf16, S = tokens_per_core/2.
      Rows 0-47 = 48 features (16 bones x 3 coords) of token set A (first
      half), row 48 = ones (bakes b1 into layer 1), rows 49-96 = set B,
      row 97 = ones. Column j holds the token pair (A_j, B_j).
  - Groups 0-15 run on the device; group 16 (the ragged 17th, a 6->16->3
    MLP = ~6% of the FLOPs) is computed on the host in numpy, which makes
    the device schedule perfectly regular: every matmul is a 32-PE-column
    unit with identical tile config (128,32), 20 units per block = exactly
    5 rounds of N=512 across the four PE column groups.
  - Layer 1 (h = relu(x @ W1 + b1)): 16 column units (4 passes x 4).
  - Layer 2 (out = h @ W2 + b2): four 32-column streams, software-pipelined
    one block behind L1 so their semaphore waits are long satisfied.
  - PSUM: pass-0/1 banks are single but evacuated in short per-pass ops;
    the pass-2/3 pair and the L2 bank are double-buffered. This breaks
    every write-after-read cycle that previously paced the pipeline.
  - The out evacuation alternates ACT/DVE by block parity to balance the
    two PSUM-capable engines.
  - Output leaves the device feature-major in bf16 (tolerance is 2e-2;
    measured error ~2.9e-3); the host transposes/casts back to f32.
"""

import sys

import numpy as np
import ml_dtypes

sys.path.insert(0, "/opt/trn_rl_repo")

import concourse.bass as bass
import concourse.mybir as mybir
import concourse.tile as tile
from concourse import bacc
from concourse.bass_utils import run_bass_kernel_spmd

BF16 = mybir.dt.bfloat16
F32 = mybir.dt.float32
BF16_NP = ml_dtypes.bfloat16

LIMBS = [[0, 1, 2], [3, 4, 5], [6, 7], [8, 9], [10, 11, 12], [13, 14, 15],
         [6, 7, 1, 2], [6, 7, 4, 5], [6, 7, 11, 12], [6, 7, 14, 15], [6, 7, 9],
         [14, 15, 11, 12], [1, 2, 4, 5], [14, 15, 4, 5], [11, 12, 4, 5],
         [10, 0], [13, 3]]
NG = 17          # groups
HID = 16         # hidden per group
B, T, NJ, C = 2048, 243, 16, 3
NF = NJ * C      # 48 input features per token
NCORES = 8
BC = B // NCORES           # batches per core
TC = BC * T                # tokens per core
S = TC // 2                # token pairs per core (2-set packing)
KX = 2 * (NF + 1)          # 98: two sets of (48 features + ones row)
NBLK = 512                 # token-pairs per block (psum free dim)
NB = (S + NBLK - 1) // NBLK   # 61 blocks (60x512 + 1x384)

# L2 stream order across PSUM quarters of the `op` bank: stream q covers
# GROUPS_L2[q], reading h of L1 pass PASS_OF_STREAM[q] from the prev block.
GROUPS_L2 = [(12, 4), (0, 4), (4, 4), (8, 4)]
PASS_OF_STREAM = [3, 0, 1, 2]


def _host_weights(W1, b1, W2, b2, idx):
    """Build stationary operands + evac bias vectors on the host.

    Returns (wsb [128, 640] bf16, bsb [128, 1] f32).
      wsb cols 0-511: L1 passes 0-3 ([98,128] each: rows 0-47 set A block,
        row 48 = set A b1, rows 49-96 set B block, row 97 = set B b1).
      wsb cols 512-639: L2 streams q=0..3 ([128,32] each).
      bsb col 0: b2 for the L2 psum bank (per-partition).
    """
    W1 = np.asarray(W1, np.float32)
    b1 = np.asarray(b1, np.float32)
    W2 = np.asarray(W2, np.float32)
    b2 = np.asarray(b2, np.float32)
    idx = np.asarray(idx)

    # Scatter per-group [12, 16] W1 blocks into the 48-feature space.
    # Padded limb rows of W1 are already zero, so += handles duplicates.
    w1full = np.zeros((NF, NG * HID), np.float32)
    for g in range(NG):
        for j in range(4):
            r = int(idx[g, j]) * C
            w1full[r:r + C, g * HID:(g + 1) * HID] += W1[g, j * C:(j + 1) * C, :]
    b1flat = b1.reshape(NG * HID)

    wsb = np.zeros((128, 640), np.float32)
    for w in range(4):
        blk = w1full[:, 64 * w:64 * w + 64]            # [48, 64]
        bias = b1flat[64 * w:64 * w + 64]
        wsb[0:NF, 128 * w:128 * w + 64] = blk          # set A
        wsb[NF, 128 * w:128 * w + 64] = bias
        wsb[NF + 1:2 * NF + 1, 128 * w + 64:128 * w + 128] = blk   # set B
        wsb[2 * NF + 1, 128 * w + 64:128 * w + 128] = bias
    for q, (g0, ng) in enumerate(GROUPS_L2):
        col = 512 + 32 * q
        for j in range(ng):
            g = g0 + j
            wsb[16 * j:16 * j + 16, col + 3 * j:col + 3 * j + 3] = W2[g]
            wsb[64 + 16 * j:64 + 16 * j + 16,
                col + 12 + 3 * j:col + 12 + 3 * j + 3] = W2[g]

    bsb = np.zeros((128, 1), np.float32)
    for q, (g0, ng) in enumerate(GROUPS_L2):
        v = b2[g0:g0 + ng].reshape(-1)                 # 12 values
        bsb[32 * q:32 * q + 12, 0] = v
        bsb[32 * q + 12:32 * q + 24, 0] = v

    return wsb.astype(BF16_NP), bsb


def _build_nc():
    nc = bacc.Bacc(
        "TRN2", target_bir_lowering=False, debug=False, num_devices=NCORES,
    )
    x2 = nc.dram_tensor("x2", [KX, S], BF16, kind="ExternalInput").ap()
    wsd = nc.dram_tensor("wsd", [128, 640], BF16, kind="ExternalInput").ap()
    bsd = nc.dram_tensor("bsd", [128, 1], F32, kind="ExternalInput").ap()
    # Device output, feature-major bf16 = the L2 psum bank layout
    # (quarter q rows 32q..32q+24 real, 12 set-A cols then 12 set-B).
    outd = nc.dram_tensor("outd", [128, S], BF16, kind="ExternalOutput").ap()

    with tile.TileContext(nc) as tc:
        with (
            tc.tile_pool(name="singles", bufs=1) as singles,
            tc.tile_pool(name="xin", bufs=4) as xin,
            tc.tile_pool(name="hsb", bufs=2) as hsb,
            tc.tile_pool(name="osb", bufs=3) as osb,
            tc.tile_pool(name="hps", bufs=1, space="PSUM") as hps,
            tc.tile_pool(name="ops", bufs=1, space="PSUM") as opsp,
        ):
            ws = singles.tile([128, 640], BF16)
            nc.sync.dma_start(ws, wsd)
            bs = singles.tile([128, 1], F32)
            nc.sync.dma_start(bs, bsd)

            h_prev = None       # (h0s, h1s, h23s) sbuf tiles of prev block
            nb_prev = 0
            xtd = None

            for s in range(NB + 1):
                cur = s if s < NB else None
                prev = s - 1 if s >= 1 else None
                if cur is not None:
                    off = cur * NBLK
                    nb = min(NBLK, S - off)
                    if s % 2 == 0:
                        # one DMA covers this block and the next
                        ld = min(2 * NBLK, S - off)
                        xtd = xin.tile([KX, 2 * NBLK], BF16, tag="xt")
                        nc.sync.dma_start(xtd[:, :ld], x2[:, off:off + ld])
                        xt = xtd[:, 0:NBLK]
                    else:
                        xt = xtd[:, NBLK:2 * NBLK]

                # ---- PE: 32-col units, round-robin over column groups ----
                if cur is not None:
                    hp0 = hps.tile([128, NBLK], F32, tag="hp0", bufs=2)
                    hp1 = hps.tile([128, NBLK], F32, tag="hp1")
                    hp23 = hps.tile([128, 2, NBLK], F32, tag="hp23", bufs=2)
                    # pass 1 first: its evac is DVE's first op of the step
                    for w in [1, 0, 2, 3]:
                        for j in range(4):
                            dst = (hp0 if w == 0 else hp1)[
                                32 * j:32 * j + 32, :nb] if w < 2 else \
                                hp23[32 * j:32 * j + 32, w % 2, :nb]
                            nc.tensor.matmul(
                                dst,
                                lhsT=ws[0:KX,
                                        128 * w + 32 * j:128 * w + 32 * j + 32],
                                rhs=xt[:, :nb],
                                start=True, stop=True,
                                tile_position=(0, 32 * j),
                            )
                if prev is not None:
                    h0p, h1p, h23p = h_prev
                    rhs_of = [h23p[:, 1], h0p, h1p, h23p[:, 0]]
                    op = opsp.tile([128, NBLK], F32, tag="op")
                    # streams reading the late-evacuated h23 pair go last
                    for q in [1, 2, 0, 3]:
                        nc.tensor.matmul(
                            op[32 * q:32 * q + 32, :nb_prev],
                            lhsT=ws[0:128, 512 + 32 * q:512 + 32 * q + 32],
                            rhs=rhs_of[q][:, :nb_prev],
                            start=True, stop=True,
                            tile_position=(0, 32 * q),
                        )

                # ---- evacuations ----
                if cur is not None:
                    h0s = hsb.tile([128, NBLK], BF16, tag="h0")
                    h1s = hsb.tile([128, NBLK], BF16, tag="h1")
                    h23s = hsb.tile([128, 2, NBLK], BF16, tag="h23")
                    # DVE: short h1 op first, then the long h23 pair;
                    # ACT: h0 relu then the out identity.
                    nc.vector.tensor_scalar(
                        h1s[:, :nb], hp1[:, :nb], 0.0, None,
                        mybir.AluOpType.max,
                    )
                    nc.scalar.activation(
                        out=h0s[:, :nb], in_=hp0[:, :nb],
                        func=mybir.ActivationFunctionType.Relu,
                    )
                    nc.vector.tensor_scalar(
                        h23s[:, :, :nb], hp23[:, :, :nb], 0.0, None,
                        mybir.AluOpType.max,
                    )
                    hts = (h0s, h1s, h23s)
                else:
                    hts = None
                if prev is not None:
                    ost = osb.tile([128, NBLK], BF16, tag="os")
                    nc.scalar.activation(
                        out=ost[:, :nb_prev], in_=op[:, :nb_prev],
                        func=mybir.ActivationFunctionType.Identity,
                        bias=bs[:, 0:1], scale=1.0,
                    )
                    offp = prev * NBLK
                    nc.sync.dma_start(
                        outd[0:128, offp:offp + nb_prev], ost[:, :nb_prev])

                h_prev = hts
                if cur is not None:
                    nb_prev = nb
    nc.finalize()
    return nc


_NC_CACHE = None


def _get_nc():
    global _NC_CACHE
    if _NC_CACHE is None:
        _NC_CACHE = _build_nc()
    return _NC_CACHE


# outd row map (see _build_nc): stream q of GROUPS_L2 at rows 32q..32q+24
# (12 set-A cols then 12 set-B).  In group order 0..15:
_ROWS_A = np.r_[32:44, 64:76, 96:108, 0:12]
_ROWS_B = np.r_[44:56, 76:88, 108:120, 12:24]


def _g16_host(x, W1, b1, W2, b2, idx):
    """Group 16 (a 6->16->3 MLP, ~6% of the FLOPs) in numpy fp32."""
    g = x[:, :, idx[16], :].reshape(B, T, 4 * C)[:, :, :2 * C]
    h = np.maximum(g @ W1[16, :2 * C].astype(np.float32)
                   + b1[16].astype(np.float32), 0.0)
    return h @ W2[16].astype(np.float32) + b2[16].astype(np.float32)


def _kernel_impl(x, W1, b1, W2, b2, idx, _want_trace=False):
    x = np.asarray(x, np.float32)
    idx = np.asarray(idx)
    wsb, bsb = _host_weights(W1, b1, W2, b2, idx)

    in_maps = []
    for c in range(NCORES):
        xc = x[c * BC:(c + 1) * BC].reshape(TC, NF)
        xt2 = np.empty((KX, S), BF16_NP)
        xt2[0:NF] = np.ascontiguousarray(xc[:S].T)
        xt2[NF] = np.float32(1.0)
        xt2[NF + 1:2 * NF + 1] = np.ascontiguousarray(xc[S:].T)
        xt2[2 * NF + 1] = np.float32(1.0)
        in_maps.append({"x2": xt2, "wsd": wsb, "bsd": bsb})

    nc = _get_nc()
    res = run_bass_kernel_spmd(
        nc, in_maps, core_ids=list(range(NCORES)), trace=_want_trace,
    )

    out = np.empty((B, T, NG, C), np.float32)
    out[:, :, 16, :] = _g16_host(x, W1, b1, W2, b2, idx)
    for c in range(NCORES):
        od = np.asarray(res.results[c]["outd"], dtype=np.float32)  # [128, S]
        oc = np.empty((TC, 16 * C), np.float32)
        oc[:S] = od[_ROWS_A].T
        oc[S:] = od[_ROWS_B].T
        out[c * BC:(c + 1) * BC, :, 0:16, :] = oc.reshape(BC, T, 16, C)
    return out, res


def kernel(**inputs):
    out, _ = _kernel_impl(**inputs)
    return out
